# revision 5
# baseline (speedup 1.0000x reference)
"""Bass/Trainium2 SPMD kernel for nn_Block3D (8 NeuronCores) — v2.

Spatial z-shard (3 own planes/core, halo recompute). Conv buffers use a padded
layout: 26-col rows (x-pad), 25-row z-groups (y-pad row between planes), one
leading pad row; each 3x3x3 tap is then a single 2-D strided slice with exact
zero padding. PE conv tiles: 27 diag matmuls per 288-col half-plane block.
DVE conv tiles: one big [rows,24] mul+add per tap. mod folds into the dynamic
kernels (no mv pass); kernel_net GEMV2 is column-form; per-voxel LN stats are
broadcast via ones-row matmuls and applied per 288-col chunk (LN3 in-place).
"""

from contextlib import ExitStack

import numpy as np
import ml_dtypes

import concourse.bass as bass
import concourse.bacc as bacc
import concourse.tile as tile
from concourse import mybir
from concourse.bass_utils import run_bass_kernel_spmd

BF = ml_dtypes.bfloat16
F32 = mybir.dt.float32
BF16 = mybir.dt.bfloat16

C = 768
G = 12
GD = 64
S = 24
HID = 4 * C
KK = 27
EPS = 1e-5
NCORES = 8
ZP = 3
PL = S * S
VC = ZP * PL              # 1728
CT = C // 128             # 6
HT = HID // 128           # 24
W1R = HID // NCORES       # 384
KFLAT = C * KK            # 20736
M2 = KFLAT // 128         # 162 m-tiles for kp2 col-form
M2C = 27                  # kp2 chunks (6 m-tiles each)

ROWS7 = 1 + 7 * 25
ROWS5 = 1 + 5 * 25
ROWS3 = 1 + 3 * 25
N7 = ROWS7 * 26           # 4576
N5 = ROWS5 * 26           # 3276
N3 = ROWS3 * 26           # 1976

# engine placement (tunable)
PE_DYN = frozenset((0, 1, 2, 3))
PE_MLP = frozenset(tuple(range(0, 12)) + (23,))
AD_MLP = frozenset()   # Act-mul + DVE-add conv
# Wi psum evacuation round-robin: 0=Act 2=Pool
EVAC_RR = (0, 0, 0, 0, 0, 0, 0, 0, 0, 0)

TAPS = [(dz, dy, dx) for dz in (-1, 0, 1) for dy in (-1, 0, 1) for dx in (-1, 0, 1)]

_CACHE = {}

Copy = mybir.ActivationFunctionType.Copy
Iden = mybir.ActivationFunctionType.Identity
Gelu = mybir.ActivationFunctionType.Gelu
Sigmoid = mybir.ActivationFunctionType.Sigmoid
Square = mybir.ActivationFunctionType.Square
Sqrt = mybir.ActivationFunctionType.Sqrt
Relu = mybir.ActivationFunctionType.Relu
ADD = mybir.AluOpType.add
SUB = mybir.AluOpType.subtract
MULT = mybir.AluOpType.mult

(S_Y, S_LB1, S_LB2, S_TPB, S_ABV, S_ABO, S_OPB, S_N2W, S_N2B, S_N3W, S_N3B,
 S_GNG, S_GNB, S_MODB, S_MLO, S_MHI) = range(16)

PE_MLP_LIST = sorted(PE_MLP)
PE_MLP_IDX = {t: i for i, t in enumerate(PE_MLP_LIST)}


def build_program():
    nc = bacc.Bacc("TRN2", target_bir_lowering=False)

    def dram_in(name, shape, dtype=F32):
        return nc.declare_dram_parameter(name, list(shape), dtype, isOutput=False)

    xpad = dram_in("xpad", [C, N7], BF16)
    smalls = dram_in("smalls", [C, 16])
    knb1 = dram_in("knb1", [W1R])
    knb2 = dram_in("knb2", [C, KK])
    gind6 = dram_in("gind6", [CT, 128, G])
    gindT6 = dram_in("gindT6", [CT, G, 128])
    ident = dram_in("ident", [128, 128], BF16)
    onesc = dram_in("onesc", [128, 1], BF16)
    loraW1T = dram_in("loraW1T", [C, C], BF16)
    modWT = dram_in("modWT", [C, C], BF16)
    modafT = dram_in("modafT", [C, C], BF16)
    w1sT = dram_in("w1sT", [C, W1R], BF16)
    w1afT = dram_in("w1afT", [C, W1R], BF16)
    w2c = dram_in("w2c", [M2C, 128, 6 * 3 * 128], BF16)
    opT = dram_in("opT", [C, C], BF16)
    wiT = dram_in("wiT", [HT, 128, CT * 128], BF16)
    woT = dram_in("woT", [CT, 128, (HID // 2 // 128) * 128], BF16)
    dmlp = dram_in("dmlp", [len(PE_MLP_LIST), 128, KK * 128], BF16)
    dwk_in = dram_in("dwk", [HID, KK])
    out = nc.declare_dram_parameter("out", [C, VC], F32, isOutput=True)

    with tile.TileContext(nc) as tc, ExitStack() as ctx:
        dram = ctx.enter_context(tc.tile_pool(name="dram", bufs=1, space="DRAM"))
        persist = ctx.enter_context(tc.tile_pool(name="persist", bufs=1))
        ytp = ctx.enter_context(tc.tile_pool(name="ytp", bufs=1))
        xbp = ctx.enter_context(tc.tile_pool(name="xbp", bufs=1))
        es1 = ExitStack()           # pools live through F1
        es_g = ExitStack()          # gate pool: G .. H
        gpool = es1.enter_context(tc.tile_pool(name="gemv", bufs=2))

        # ---------- persistent inputs ----------
        xsp = es1.enter_context(tc.tile_pool(name="xsp", bufs=1))
        xs = [xsp.tile([128, N7], BF16, name=f"xs{i}", tag=f"xs{i}")
              for i in range(CT)]
        for i in range(CT):
            nc.sync.dma_start(xs[i][:], xpad[128 * i:128 * (i + 1), :])
        sm = [persist.tile([128, 16], F32, name=f"sm{i}", tag=f"sm{i}")
              for i in range(CT)]
        for i in range(CT):
            nc.sync.dma_start(sm[i][:], smalls[128 * i:128 * (i + 1), :])
        id_t = persist.tile([128, 128], BF16, name="identt", tag="identt")
        nc.sync.dma_start(id_t[:], ident[:, :])
        ones_t = persist.tile([128, 1], BF16, name="onest", tag="onest")
        nc.sync.dma_start(ones_t[:], onesc[:, :])
        onesr = persist.tile([1, 128], BF16, name="onesr", tag="onesr")
        nc.vector.memset(onesr[:], 1.0)
        eps_t = persist.tile([128, 1], F32, name="epst", tag="epst")
        nc.vector.memset(eps_t[:], EPS)
        junk = persist.tile([128, 676], BF16, name="junk", tag="junk")
        dwkb = persist.tile([128, HT * KK], F32, name="dwkb", tag="dwkb")

        # xb + dyn/xln (aliased) are ctx-lifetime
        xb = [xbp.tile([128, N5], BF16, name=f"xb{i}", tag=f"xb{i}")
              for i in range(CT)]
        dyn = [xbp.tile([128, N5], BF16, name=f"dyn{i}", tag=f"dyn{i}")
               for i in range(CT)]

        # LN2 scale/shift columns; variants 0=interior 1=low-edge 2=high-edge
        n2wv = [[sm[i][:, S_N2W:S_N2W + 1] for i in range(CT)]]
        n2bv = [[sm[i][:, S_N2B:S_N2B + 1] for i in range(CT)]]
        for svar in (S_MLO, S_MHI):
            wl, bl = [], []
            for i in range(CT):
                w = persist.tile([128, 1], F32, name=f"nw{svar}{i}",
                                 tag=f"nw{svar}{i}")
                nc.vector.tensor_mul(w[:], sm[i][:, S_N2W:S_N2W + 1],
                                     sm[i][:, svar:svar + 1])
                b = persist.tile([128, 1], F32, name=f"nb{svar}{i}",
                                 tag=f"nb{svar}{i}")
                nc.vector.tensor_mul(b[:], sm[i][:, S_N2B:S_N2B + 1],
                                     sm[i][:, svar:svar + 1])
                wl.append(w[:, 0:1])
                bl.append(b[:, 0:1])
            n2wv.append(wl)
            n2bv.append(bl)

        def r26(t, rows):
            return t.rearrange("p (r x) -> p r x", r=rows, x=26)

        # ---------- conv helpers (padded layout) ----------
        def conv_pe(dst_fn, src, srows, dg, planes, delta, psum_pool):
            s3 = r26(src, srows)
            for d in range(planes):
                for hb in range(2):
                    ps = psum_pool.tile([128, 288], F32, name="cvps",
                                        tag="cvps")
                    for ti, (dz, dy, dx) in enumerate(TAPS):
                        r0 = 1 + 25 * (d + delta + dz) + 12 * hb + dy
                        sv = s3[:, r0:r0 + 12, 1 + dx:25 + dx]
                        nc.tensor.matmul(ps[:], dg[:, 128 * ti:128 * (ti + 1)],
                                         sv, start=(ti == 0),
                                         stop=(ti == KK - 1))
                    dst_fn(d, hb, ps)

        def conv_dve(dst, drows, src, srows, ktile, planes, delta, tmp_pool,
                     kofs=0, mode="dve"):
            NR = 25 * planes - 1
            d3 = r26(dst, drows)
            s3 = r26(src, srows)
            ov = d3[:, 1:1 + NR, 1:25]
            for ti, (dz, dy, dx) in enumerate(TAPS):
                r0 = 1 + 25 * (delta + dz) + dy
                sv = s3[:, r0:r0 + NR, 1 + dx:25 + dx]
                sc = ktile[:, kofs + ti:kofs + ti + 1]
                if ti == 0:
                    if mode == "ad":
                        nc.scalar.activation(ov, sv, Copy, scale=sc)
                    else:
                        nc.vector.tensor_scalar_mul(ov, sv, sc)
                else:
                    tmp = tmp_pool.tile([128, drows * 26], BF16, name="cvt",
                                        tag="cvt")
                    t3 = r26(tmp, drows)
                    tv = t3[:, 1:1 + NR, 1:25]
                    if mode == "ad":
                        nc.scalar.activation(tv, sv, Copy, scale=sc)
                    else:
                        nc.vector.tensor_scalar_mul(tv, sv, sc)
                    nc.vector.tensor_tensor(ov, ov, tv, ADD)

        # ---------- per-voxel LN stats + broadcast (per 288-col chunk) ----
        def make_lnstats(tag, es):
            lps = es.enter_context(
                tc.tile_pool(name=f"{tag}ps", bufs=1, space="PSUM"))
            sqp = es.enter_context(tc.tile_pool(name=f"{tag}sq", bufs=2))
            smp = es.enter_context(tc.tile_pool(name=f"{tag}sm", bufs=2))
            bkp = es.enter_context(tc.tile_pool(name=f"{tag}bk", bufs=3))

            def chunk(t3s, r0):
                n = 288
                ps1 = lps.tile([1, 288], F32, name="s1", tag="s1")
                ps2 = lps.tile([1, 288], F32, name="s2", tag="s2")
                for k in range(CT):
                    nc.tensor.matmul(ps1[:, 0:n], ones_t[:],
                                     t3s[k][:, r0:r0 + 12, 1:25],
                                     start=(k == 0), stop=(k == CT - 1))
                for k in range(CT):
                    sq = sqp.tile([128, 288], BF16, name="sq", tag="sq")
                    s3v = sq.rearrange("p (a b) -> p a b", a=12, b=24)
                    nc.scalar.activation(s3v[:, :, :],
                                         t3s[k][:, r0:r0 + 12, 1:25], Square)
                    nc.tensor.matmul(ps2[:, 0:n], ones_t[:], sq[:, 0:n],
                                     start=(k == 0), stop=(k == CT - 1))
                mu_f = smp.tile([1, 288], F32, name="muf", tag="muf")
                nc.scalar.activation(mu_f[:], ps1[:, 0:n], Copy, scale=1.0 / C)
                murow = smp.tile([1, 288], BF16, name="murow", tag="murow")
                nc.scalar.activation(murow[:], ps1[:, 0:n], Copy,
                                     scale=1.0 / C)
                m2 = smp.tile([1, 288], F32, name="m2f", tag="m2f")
                nc.scalar.activation(m2[:], ps2[:, 0:n], Copy, scale=1.0 / C)
                vr = smp.tile([1, 288], F32, name="vrf", tag="vrf")
                nc.vector.tensor_tensor(vr[:], mu_f[:], mu_f[:], MULT)
                nc.vector.tensor_sub(vr[:], m2[:], vr[:])
                nc.scalar.activation(vr[:], vr[:], Sqrt, bias=eps_t[0:1, 0:1])
                nc.vector.reciprocal(vr[:], vr[:])
                rsrow = smp.tile([1, 288], BF16, name="rsrow", tag="rsrow")
                nc.scalar.activation(rsrow[:], vr[:], Copy)
                # broadcast rows to all partitions via ones-row matmul
                bp1 = lps.tile([128, 288], F32, name="bp1", tag="bp1")
                nc.tensor.matmul(bp1[:], onesr[:], murow[:],
                                 start=True, stop=True)
                mublk = bkp.tile([128, 288], BF16, name="mublk", tag="mublk")
                nc.scalar.activation(mublk[:], bp1[:], Copy)
                bp2 = lps.tile([128, 288], F32, name="bp2", tag="bp2")
                nc.tensor.matmul(bp2[:], onesr[:], rsrow[:],
                                 start=True, stop=True)
                rsblk = bkp.tile([128, 288], BF16, name="rsblk", tag="rsblk")
                nc.scalar.activation(rsblk[:], bp2[:], Copy)
                return mublk, rsblk
            return chunk

        # ---- phase A: vc partial sums (own-plane windows) + AllGather ----
        vcs = persist.tile([128, 3 * CT], F32, name="vcs", tag="vcs")
        vsum = persist.tile([128, CT], F32, name="vsum", tag="vsum")
        jw = junk.rearrange("p (a b) -> p a b", a=26, b=26)
        for i in range(CT):
            x3 = r26(xs[i], ROWS7)
            for d in range(3):
                r0 = 1 + 25 * (2 + d)
                nc.scalar.activation(jw[:, 0:24, 0:24],
                                     x3[:, r0:r0 + 24, 1:25], Copy,
                                     accum_out=vcs[:, 3 * i + d:3 * i + d + 1])
            nc.scalar.activation(junk[:, 0:3], vcs[:, 3 * i:3 * i + 3], Copy,
                                 accum_out=vsum[:, i:i + 1])
        ag1_in = dram.tile([C], F32, name="ag1i", tag="ag1i")
        ag1_out = dram.tile([NCORES, C], F32, name="ag1o", tag="ag1o",
                            addr_space="Shared")
        nc.sync.dma_start(
            bass.AP(tensor=ag1_in[:].tensor, offset=ag1_in[:].offset,
                    ap=[[1, 128], [128, CT]]), vsum[:])
        nc.gpsimd.collective_compute(
            "AllGather", mybir.AluOpType.bypass,
            replica_groups=[list(range(NCORES))],
            ins=[ag1_in[:]], outs=[ag1_out[:]])

        # ---- phase B-D: text branch, mod, kernel_net ----
        es_bd = ExitStack()
        es_pe = ExitStack()
        psA = es_pe.enter_context(
            tc.tile_pool(name="psA", bufs=2, space="PSUM"))
        stw = es_bd.enter_context(tc.tile_pool(name="stw", bufs=2))

        def gemv(wdram, wcols, in_cols, nk, nm, act, bias_cols, tag,
                 scale=1.0, odt=BF16, opool=None, wrow0=0):
            wb = stw.tile([128, 6 * 768], BF16, name="stw", tag="stw")
            nc.sync.dma_start(
                wb[:, 0:nk * wcols],
                bass.AP(tensor=wdram, offset=wrow0 * wcols,
                        ap=[[wcols, 128], [128 * wcols, nk], [1, wcols]]))
            outs = []
            for m in range(nm):
                ps = psA.tile([128, 1], F32, name="ps_small", tag="ps_small")
                for k in range(nk):
                    c0 = k * wcols + 128 * m
                    nc.tensor.matmul(ps[:], wb[:, c0:c0 + 128],
                                     in_cols[k][:],
                                     start=(k == 0), stop=(k == nk - 1))
                o = (opool or gpool).tile([128, 1], odt, name=f"{tag}o{m}",
                                          tag=f"{tag}o{m}")
                bias = bias_cols[m] if bias_cols is not None else 0.0
                nc.scalar.activation(o[:], ps[:], act, bias=bias, scale=scale)
                outs.append(o)
            return outs

        t_cols = []
        for i in range(CT):
            t = gpool.tile([128, 1], BF16, name=f"tc{i}", tag=f"tc{i}")
            nc.scalar.activation(t[:], sm[i][:, S_Y:S_Y + 1], Copy)
            t_cols.append(t)
        h1 = gemv(loraW1T, C, t_cols, CT, CT, Relu,
                  [sm[i][:, S_LB1:S_LB1 + 1] for i in range(CT)], "lw1")

        # attn-half partials of mod / kp1 via host-folded linear tail
        mod_a = gemv(modafT, C, h1, CT, CT, Iden,
                     [sm[i][:, S_MODB:S_MODB + 1] for i in range(CT)],
                     "moda", odt=F32)
        knb1_t = persist.tile([128, 3], F32, name="knb1t", tag="knb1t")
        nc.sync.dma_start(
            knb1_t[:],
            bass.AP(tensor=knb1, offset=0, ap=[[1, 128], [128, 3]]))
        kp1_a = gemv(w1afT, W1R, h1, CT, 3, Iden,
                     [knb1_t[:, m:m + 1] for m in range(3)], "w1a",
                     odt=F32)

        # vc columns from AllGather
        comb = []
        for i in range(CT):
            red = gpool.tile([128, NCORES], F32, name=f"red{i}",
                             tag=f"red{i}")
            nc.sync.dma_start(
                red[:], bass.AP(tensor=ag1_out[:].tensor,
                                offset=ag1_out[:].offset + 128 * i,
                                ap=[[1, 128], [C, NCORES]]))
            cb = gpool.tile([128, 1], BF16, name=f"cmb{i}", tag=f"cmb{i}")
            vcc = gpool.tile([128, 1], F32, name=f"vcc{i}", tag=f"vcc{i}")
            nc.scalar.activation(junk[:, 0:NCORES], red[:], Copy,
                                 scale=1.0 / (S * S * S), accum_out=vcc[:])
            nc.scalar.activation(cb[:], vcc[:], Copy)
            comb.append(cb)

        mod = gemv(modWT, C, comb, CT, CT, Sigmoid,
                   [mod_a[m][:, 0:1] for m in range(CT)],
                   "modw", odt=F32, opool=persist)
        kp1 = gemv(w1sT, W1R, comb, CT, 3, Relu,
                   [kp1_a[m][:, 0:1] for m in range(3)], "w1s")
        del h1

        # kp2: column-form — stationary w2 blocks, moving kp1 columns
        ar2_in = dram.tile([KFLAT], F32, name="ar2i", tag="ar2i")
        ar2_out = dram.tile([KFLAT], F32, name="ar2o", tag="ar2o",
                            addr_space="Shared")
        k2loc = persist.tile([128, M2], F32, name="k2loc", tag="k2loc")
        with (tc.tile_pool(name="w2str", bufs=2) as w2str,
              tc.tile_pool(name="kps", bufs=2, space="PSUM") as kps):
            for ch in range(M2C):
                wt = w2str.tile([128, 6 * 3 * 128], BF16, name="w2ch",
                                tag="w2ch")
                nc.sync.dma_start(wt[:], w2c[ch, :, :])
                ps = kps.tile([128, 6], F32, name="kpsb", tag="kpsb")
                for m6 in range(6):
                    for k in range(3):
                        c0 = (m6 * 3 + k) * 128
                        nc.tensor.matmul(ps[:, m6:m6 + 1],
                                         wt[:, c0:c0 + 128], kp1[k][:],
                                         start=(k == 0), stop=(k == 2))
                nc.scalar.activation(k2loc[:, 6 * ch:6 * ch + 6], ps[:], Copy)
        nc.sync.dma_start(ar2_in[:], k2loc[:])
        nc.gpsimd.collective_compute(
            "AllReduce", ADD, replica_groups=[list(range(NCORES))],
            ins=[ar2_in[:]], outs=[ar2_out[:]])

        # kern tiles: contiguous gather + bias, fold mod
        kern = []
        for i in range(CT):
            kt = persist.tile([128, KK], F32, name=f"kern{i}", tag=f"kern{i}")
            nc.sync.dma_start(
                kt[:], bass.AP(tensor=ar2_out[:].tensor,
                               offset=ar2_out[:].offset + 128 * KK * i,
                               ap=[[KK, 128], [1, KK]]))
            kb = gpool.tile([128, KK], F32, name=f"kernb{i}", tag=f"kernb{i}")
            nc.sync.dma_start(kb[:], knb2[128 * i:128 * (i + 1), :])
            nc.vector.tensor_add(kt[:], kt[:], kb[:])
            nc.vector.tensor_scalar_mul(kt[:], kt[:], mod[i][:, 0:1])
            kern.append(kt)

        nc.sync.dma_start(
            dwkb[:], bass.AP(tensor=dwk_in, offset=0,
                             ap=[[KK, 128], [128 * KK, HT], [1, KK]]))
        es_bd.close()

        # ---- phase E: dynamic conv (5 planes out) + GN stats ----
        es_e = ExitStack()
        actp = es_e.enter_context(tc.tile_pool(name="actp", bufs=1))
        opT_t = [actp.tile([128, C], BF16, name=f"opT{i}", tag=f"opT{i}")
                 for i in range(CT)]
        for i in range(CT):
            nc.sync.dma_start(opT_t[i][:], opT[128 * i:128 * (i + 1), :])
        ag3_in = dram.tile([G, 6], F32, name="ag3i", tag="ag3i")
        ag3_out = dram.tile([NCORES, G, 6], F32, name="ag3o", tag="ag3o",
                            addr_space="Shared")
        gsb = persist.tile([G, 6], F32, name="gsb", tag="gsb")
        DYN_ORDER = sorted(range(CT), key=lambda i: i in PE_DYN)
        with (tc.tile_pool(name="mvp", bufs=2) as mvp,
              tc.tile_pool(name="dgp", bufs=2) as dgp,
              tc.tile_pool(name="cvtmp", bufs=1) as cvtmp,
              tc.tile_pool(name="cpsum", bufs=3, space="PSUM") as cpsum,
              tc.tile_pool(name="gnps", bufs=1, space="PSUM") as gnps):
            gps = gnps.tile([G, 6], F32, name="gps", tag="gps")
            for pos, i in enumerate(DYN_ORDER):
                if i in PE_DYN:
                    dg = dgp.tile([128, KK * 128], BF16, name="dg", tag="dg")
                    for t in range(KK):
                        nc.scalar.activation(
                            dg[:, 128 * t:128 * (t + 1)], id_t[:], Copy,
                            scale=kern[i][:, t:t + 1] if hasattr(kern[i], 'tensor') else kern[i][:, t:t+1])
                    d3 = r26(dyn[i], ROWS5)

                    def wr(d, hb, ps, d3=d3):
                        r0 = 1 + 25 * d + 12 * hb
                        nc.scalar.activation(d3[:, r0:r0 + 12, 1:25],
                                             ps[:], Copy)
                    conv_pe(wr, xs[i], ROWS7, dg, 5, 1, cpsum)
                else:
                    conv_dve(dyn[i], ROWS5, xs[i], ROWS7, kern[i], 5, 1,
                             cvtmp)
                st = mvp.tile([128, 6], F32, name="gnst", tag="gnst")
                dd3 = r26(dyn[i], ROWS5)
                for d in range(3):
                    r0 = 1 + 25 * (d + 1)
                    win = dd3[:, r0:r0 + 24, 1:25]
                    nc.scalar.activation(jw[:, 0:24, 0:24], win, Copy,
                                         accum_out=st[:, d:d + 1])
                    sqw = mvp.tile([128, 576], BF16, name="sqw", tag="sqw")
                    sq3 = sqw.rearrange("p (a b) -> p a b", a=24, b=24)
                    nc.vector.tensor_tensor(sq3[:, :, :], win, win, MULT)
                    nc.scalar.activation(jw[:, 0:24, 0:24], sq3[:, :, :],
                                         Copy, accum_out=st[:, 3 + d:4 + d])
                gi = mvp.tile([128, G], F32, name="gind", tag="gind")
                nc.sync.dma_start(gi[:], gind6[i, :, :])
                nc.tensor.matmul(gps[:], gi[:], st[:],
                                 start=(pos == 0), stop=(pos == CT - 1))
            nc.scalar.activation(gsb[:], gps[:], Copy)
        nc.sync.dma_start(ag3_in[:], gsb[:])
        nc.gpsimd.collective_compute(
            "AllGather", mybir.AluOpType.bypass,
            replica_groups=[list(range(NCORES))],
            ins=[ag3_in[:]], outs=[ag3_out[:]])
        gstat = persist.tile([G, 2], F32, name="gstat", tag="gstat")
        for s in range(2):
            rg = gpool.tile([G, 24], F32, name=f"rg{s}", tag=f"rg{s}")
            nc.sync.dma_start(
                rg[:], bass.AP(tensor=ag3_out[:].tensor,
                               offset=ag3_out[:].offset + 3 * s,
                               ap=[[6, G], [6 * G, NCORES], [1, 3]]))
            nc.scalar.activation(junk[0:G, 0:24], rg[:], Copy,
                                 accum_out=gstat[:, s:s + 1])
        NGRP = float(GD * S * S * S)
        gmr = persist.tile([G, 2], F32, name="gmr", tag="gmr")
        nc.scalar.activation(gmr[:, 0:1], gstat[:, 0:1], Copy,
                             scale=1.0 / NGRP)
        musq = persist.tile([G, 1], F32, name="musq", tag="musq")
        nc.scalar.square(musq[:], gmr[:, 0:1])
        var = persist.tile([G, 1], F32, name="gvar", tag="gvar")
        nc.vector.tensor_scalar(var[:], gstat[:, 1:2], 1.0 / NGRP, None,
                                op0=MULT)
        nc.vector.tensor_sub(var[:], var[:], musq[:])
        nc.scalar.activation(var[:], var[:], Sqrt, bias=eps_t[0:G, 0:1])
        nc.vector.reciprocal(gmr[:, 1:2], var[:])

        # fold GN scale into opT, shift into bias columns
        cafm_shift = []
        gsc = []
        for i in range(CT):
            git = gpool.tile([G, 128], F32, name=f"git{i}", tag=f"git{i}")
            nc.sync.dma_start(git[:], gindT6[i, :, :])
            psg = psA.tile([128, 2], F32, name="psg", tag="psg")
            nc.tensor.matmul(psg[:], git[:], gmr[:], start=True, stop=True)
            mu_c = persist.tile([128, 2], F32, name=f"muc{i}", tag=f"muc{i}")
            nc.scalar.activation(mu_c[:], psg[:], Copy)
            a = persist.tile([128, 1], F32, name=f"gsc{i}", tag=f"gsc{i}")
            nc.vector.tensor_mul(a[:], sm[i][:, S_GNG:S_GNG + 1],
                                 mu_c[:, 1:2])
            b = persist.tile([128, 1], F32, name=f"gsh{i}", tag=f"gsh{i}")
            nc.vector.tensor_mul(b[:], mu_c[:, 0:1], a[:])
            nc.vector.tensor_sub(b[:], sm[i][:, S_GNB:S_GNB + 1], b[:])
            gsc.append(a)
            bb = gpool.tile([128, 1], BF16, name=f"gshb{i}", tag=f"gshb{i}")
            nc.scalar.activation(bb[:], b[:], Copy)
            cafm_shift.append(bb)
        cb_cols = []
        for m in range(CT):
            ps = psA.tile([128, 1], F32, name="ps_small", tag="ps_small")
            for k in range(CT):
                nc.tensor.matmul(ps[:], opT_t[k][:, 128 * m:128 * (m + 1)],
                                 cafm_shift[k][:], start=(k == 0),
                                 stop=(k == CT - 1))
            o = persist.tile([128, 1], F32, name=f"cbc{m}", tag=f"cbc{m}")
            nc.scalar.activation(o[:], ps[:], Iden,
                                 bias=sm[m][:, S_OPB:S_OPB + 1])
            cb_cols.append(o)
        for i in range(CT):
            nc.vector.tensor_scalar_mul(opT_t[i][:], opT_t[i][:], gsc[i][:])

        # ---- phase F1: cafm matmul -> xb, fused LN2 stats + apply ----
        es_pe.close()
        es_ln2 = ExitStack()
        ln2_chunk = make_lnstats("ln2", es_ln2)
        dyn3 = [r26(dyn[k], ROWS5) for k in range(CT)]
        xs3 = [r26(xs[k], ROWS7) for k in range(CT)]
        xb3 = [r26(xb[k], ROWS5) for k in range(CT)]
        with tc.tile_pool(name="opwps", bufs=4, space="PSUM") as opwps:
            for d in range(5):
                for hb in range(2):
                    r0 = 1 + 25 * d + 12 * hb
                    for m in range(CT):
                        ps = opwps.tile([128, 288], F32, name="opw",
                                        tag="opw")
                        pv = ps.rearrange("p (a b) -> p a b", a=12, b=24)
                        for k in range(CT):
                            nc.tensor.matmul(
                                ps[:], opT_t[k][:, 128 * m:128 * (m + 1)],
                                dyn3[k][:, r0:r0 + 12, 1:25],
                                start=(k == 0), stop=(k == CT - 1))
                        nc.vector.scalar_tensor_tensor(
                            xb3[m][:, r0:r0 + 12, 1:25], pv[:, :, :],
                            cb_cols[m][:],
                            xs3[m][:, r0 + 25:r0 + 37, 1:25],
                            op0=ADD, op1=MULT)
                    mublk, rsblk = ln2_chunk(xb3, r0)
                    mu3 = mublk.rearrange("p (a b) -> p a b", a=12, b=24)
                    rs3 = rsblk.rearrange("p (a b) -> p a b", a=12, b=24)
                    var_i = 1 if d == 0 else (2 if d == 4 else 0)
                    for k in range(CT):
                        xv = dyn3[k][:, r0:r0 + 12, 1:25]  # xln (aliased)
                        nc.vector.tensor_tensor(
                            xv, xb3[k][:, r0:r0 + 12, 1:25],
                            mu3[:, :, :], SUB)
                        nc.vector.tensor_tensor(xv, xv, rs3[:, :, :], MULT)
                        nc.vector.tensor_scalar(xv, xv, n2wv[var_i][k],
                                                n2bv[var_i][k],
                                                op0=MULT, op1=ADD)
        es_ln2.close()
        es_e.close()
        es1.close()   # free xs/gemv pools
        xln3 = dyn3
        gatep = es_g.enter_context(tc.tile_pool(name="gatep", bufs=1))

        # ---- phase G: MLP ----
        with tc.tile_pool(name="hpbp", bufs=1) as hpbp:
            hpb = [hpbp.tile([128, N5], BF16, name=f"hp{k}", tag=f"hp{k}")
                   for k in range(2)]
            for k in range(2):
                nc.gpsimd.memset(hpb[k][:], 0.0)
            gate = [gatep.tile([128, N3], BF16, name=f"gate{j}",
                               tag=f"gate{j}") for j in range(HT // 2)]
            with (tc.tile_pool(name="wiw", bufs=2) as wiw,
                  tc.tile_pool(name="diag", bufs=2) as dpool,
                  tc.tile_pool(name="glueG", bufs=2) as glueG,
                  tc.tile_pool(name="cvtmp2", bufs=2) as cvtmp2,
                  tc.tile_pool(name="wips", bufs=4, space="PSUM") as wips,
                  tc.tile_pool(name="cvps2", bufs=3, space="PSUM") as cvps2):

                def mlp_tile(tt, hslot, dst, act):
                    wall = wiw.tile([128, CT * 128], BF16, name="wiall",
                                    tag="wiall")
                    nc.sync.dma_start(wall[:], wiT[tt, :, :])
                    hp = hpb[hslot]
                    h3 = r26(hp, ROWS5)
                    for d in range(5):
                        for hb in range(2):
                            r0 = 1 + 25 * d + 12 * hb
                            ps = wips.tile([128, 288], F32, name="wi_ps",
                                           tag="wi_ps")
                            pv = ps.rearrange("p (a b) -> p a b", a=12, b=24)
                            for k in range(CT):
                                nc.tensor.matmul(
                                    ps[:], wall[:, 128 * k:128 * (k + 1)],
                                    xln3[k][:, r0:r0 + 12, 1:25],
                                    start=(k == 0), stop=(k == CT - 1))
                            ev = EVAC_RR[(2 * d + hb) % len(EVAC_RR)]
                            hv = h3[:, r0:r0 + 12, 1:25]
                            if ev == 0:
                                nc.scalar.activation(hv, pv[:, :, :], Copy)
                            else:
                                nc.vector.tensor_copy(hv, pv[:, :, :])
                    d3o = r26(dst, ROWS3)
                    if tt in PE_MLP:
                        dga = dpool.tile([128, KK * 128], BF16, name="dgall",
                                         tag="dgall")
                        nc.sync.dma_start(dga[:], dmlp[PE_MLP_IDX[tt], :, :])

                        def wr(d, hb, ps):
                            r0 = 1 + 25 * d + 12 * hb
                            nc.scalar.activation(d3o[:, r0:r0 + 12, 1:25],
                                                 ps[:], act)
                        conv_pe(wr, hp, ROWS5, dga, 3, 1, cvps2)
                    else:
                        conv_dve(dst, ROWS3, hp, ROWS5, dwkb, 3, 1, cvtmp2,
                                 kofs=KK * tt,
                                 mode="ad" if tt in AD_MLP else "dve")
                        if act is Gelu:
                            dv = d3o[:, 1:75, 1:25]
                            nc.scalar.activation(dv, dv, Gelu)

                for j in range(HT // 2):
                    g1 = glueG.tile([128, N3], BF16, name="gelu1",
                                    tag="gelu1")
                    mlp_tile(j, 0, g1, Gelu)
                    c2 = glueG.tile([128, N3], BF16, name="conv2",
                                    tag="conv2")
                    mlp_tile(j + HT // 2, 1, c2, Copy)
                    g13 = r26(g1, ROWS3)
                    c23 = r26(c2, ROWS3)
                    gg3 = r26(gate[j], ROWS3)
                    nc.gpsimd.tensor_tensor(gg3[:, 1:75, 1:25],
                                            g13[:, 1:75, 1:25],
                                            c23[:, 1:75, 1:25], MULT)

        # ---- phase H: Wo + residual, fused LN3 stats; LN3 in place ----
        y_t = [ytp.tile([128, N3], BF16, name=f"y{i}", tag=f"y{i}")
               for i in range(CT)]
        es_ln3 = ExitStack()
        ln3_chunk = make_lnstats("ln3", es_ln3)
        gate3 = [r26(gate[k], ROWS3) for k in range(HT // 2)]
        yt3 = [r26(y_t[k], ROWS3) for k in range(CT)]
        with (tc.tile_pool(name="wow", bufs=6) as wow,
              tc.tile_pool(name="wops", bufs=3, space="PSUM") as wops,
              tc.tile_pool(name="glueH", bufs=3) as glueH):
            walls = []
            for m in range(CT):
                wall = wow.tile([128, (HT // 2) * 128], BF16, name="woall",
                                tag="woall")
                nc.sync.dma_start(wall[:], woT[m, :, :])
                walls.append(wall)
            for d in range(3):
                for hb in range(2):
                    r0 = 1 + 25 * d + 12 * hb
                    for m in range(CT):
                        ps = wops.tile([128, 288], F32, name="wo_ps",
                                       tag="wo_ps")
                        pv = ps.rearrange("p (a b) -> p a b", a=12, b=24)
                        for k in range(HT // 2):
                            nc.tensor.matmul(
                                ps[:], walls[m][:, 128 * k:128 * (k + 1)],
                                gate3[k][:, r0:r0 + 12, 1:25],
                                start=(k == 0), stop=False)
                        nc.tensor.matmul(ps[:], id_t[:],
                                         xb3[m][:, r0 + 25:r0 + 37, 1:25],
                                         start=False, stop=True)
                        nc.scalar.activation(yt3[m][:, r0:r0 + 12, 1:25],
                                             pv[:, :, :], Copy)
                    mublk, rsblk = ln3_chunk(yt3, r0)
                    mu3 = mublk.rearrange("p (a b) -> p a b", a=12, b=24)
                    rs3 = rsblk.rearrange("p (a b) -> p a b", a=12, b=24)
                    for k in range(CT):
                        yv = yt3[k][:, r0:r0 + 12, 1:25]
                        nc.vector.tensor_tensor(yv, yv, mu3[:, :, :], SUB)
                        nc.vector.tensor_tensor(yv, yv, rs3[:, :, :], MULT)
                # plane d fully normalized: final scale + out DMA per tile
                for i in range(CT):
                    of_pl = glueH.tile([128, 576], F32, name="ofpl",
                                       tag="ofpl")
                    o2 = of_pl.rearrange("p (y x) -> p y x", y=24, x=24)
                    rp = 1 + 25 * d
                    nc.scalar.activation(o2[:, :, :],
                                         yt3[i][:, rp:rp + 24, 1:25], Iden,
                                         scale=sm[i][:, S_N3W:S_N3W + 1],
                                         bias=sm[i][:, S_N3B:S_N3B + 1])
                    nc.sync.dma_start(
                        out[128 * i:128 * (i + 1), 576 * d:576 * (d + 1)],
                        of_pl[:])
        es_ln3.close()
        es_g.close()

    nc.compile()
    return nc


def _prep(inputs):
    bf = lambda a: np.ascontiguousarray(a).astype(BF)
    f32 = lambda a: np.ascontiguousarray(a, dtype=np.float32)
    x = f32(inputs["x"][0])

    smalls0 = np.zeros((C, 16), np.float32)
    smalls0[:, 0] = f32(inputs["y"][0, 0])
    for i, k in enumerate(["lora_b1", "lora_b2", "tp_b", "attn_bv", "attn_bo",
                           "op_b", "n2_w", "n2_b", "n3_w", "n3_b", "gn_g",
                           "gn_b", "mod_b"]):
        smalls0[:, i + 1] = f32(inputs[k])

    gind6 = np.zeros((CT, 128, G), np.float32)
    for j in range(CT):
        for p in range(128):
            gind6[j, p, (128 * j + p) // GD] = 1.0
    gindT6 = np.ascontiguousarray(gind6.transpose(0, 2, 1))

    kn_W1 = f32(inputs["kn_W1"])
    kn_W2 = f32(inputs["kn_W2"])
    knb2 = f32(inputs["kn_b2"]).reshape(C, KK)

    mlp_dw = f32(inputs["mlp_dw"]).reshape(HID, KK)
    dmlp = np.zeros((len(PE_MLP_LIST), 128, KK, 128), np.float32)
    idx = np.arange(128)
    for di, tt in enumerate(PE_MLP_LIST):
        for ti in range(KK):
            dmlp[di, idx, ti, idx] = mlp_dw[128 * tt:128 * (tt + 1), ti]
    dmlp = dmlp.reshape(len(PE_MLP_LIST), 128, KK * 128)

    T3 = f32(inputs["attn_Wo"]).T
    T2 = f32(inputs["attn_Wv"]).T @ T3
    T1 = f32(inputs["tp_W"]).T @ T2
    Wfold = f32(inputs["lora_W2"]).T @ T1
    battn = (f32(inputs["lora_b2"]) @ T1 + f32(inputs["tp_b"]) @ T2
             + f32(inputs["attn_bv"]) @ T3 + f32(inputs["attn_bo"]))
    mod_W = f32(inputs["mod_W"])
    modab = battn @ mod_W[:, C:].T + f32(inputs["mod_b"])
    smalls0[:, S_MODB] = modab
    com = dict(
        knb2=knb2, gind6=gind6, gindT6=gindT6,
        ident=bf(np.eye(128, dtype=np.float32)),
        onesc=bf(np.ones((128, 1), np.float32)),
        loraW1T=bf(f32(inputs["lora_W1"]).T),
        modWT=bf(mod_W[:, :C].T),
        modafT=bf(Wfold @ mod_W[:, C:].T),
        opT=bf(f32(inputs["op_W"]).T),
        wiT=bf(f32(inputs["mlp_Wi"]).T.reshape(CT, 128, HT, 128)
               .transpose(2, 1, 0, 3).reshape(HT, 128, CT * 128)),
        woT=bf(f32(inputs["mlp_Wo"]).T.reshape(HT // 2, 128, CT, 128)
               .transpose(2, 1, 0, 3).reshape(CT, 128, (HT // 2) * 128)),
        dmlp=bf(dmlp), dwk=mlp_dw,
    )

    xf = x.reshape(C, S, PL)
    in_maps = []
    for i in range(NCORES):
        z0 = ZP * i
        xp = np.zeros((C, ROWS7, 26), np.float32)
        for ls in range(7):
            g = z0 - 2 + ls
            if 0 <= g < S:
                xp[:, 1 + 25 * ls:1 + 25 * ls + 24, 1:25] = \
                    xf[:, g].reshape(C, 24, 24)
        smalls = smalls0.copy()
        smalls[:, S_MLO] = 1.0 if i > 0 else 0.0
        smalls[:, S_MHI] = 1.0 if i < NCORES - 1 else 0.0
        a = kn_W2[:, W1R * i:W1R * (i + 1)]        # [20736, 384]
        a = a.reshape(128, M2, 3, 128)             # [p, m, k, kk]
        w2ct = a.transpose(1, 2, 3, 0).reshape(M2C, 6, 3, 128, 128)
        w2ct = np.ascontiguousarray(w2ct.transpose(0, 3, 1, 2, 4)) \
            .reshape(M2C, 128, 6 * 3 * 128)
        m = dict(com)
        w1rows = kn_W1[W1R * i:W1R * (i + 1), :]
        m.update(
            xpad=xp.reshape(C, N7).astype(BF),
            smalls=smalls,
            knb1=(battn @ w1rows[:, C:].T
                  + f32(inputs["kn_b1"][W1R * i:W1R * (i + 1)])),
            w1sT=bf(w1rows[:, :C].T),
            w1afT=bf(Wfold @ w1rows[:, C:].T),
            w2c=bf(w2ct),
        )
        in_maps.append(m)
    return in_maps


def kernel(**inputs) -> np.ndarray:
    if "nc" not in _CACHE:
        _CACHE["nc"] = build_program()
    nc = _CACHE["nc"]
    in_maps = _prep(inputs)
    res = run_bass_kernel_spmd(nc, in_maps, list(range(NCORES)))
    outs = [res.results[i]["out"].reshape(C, ZP, PL) for i in range(NCORES)]
    full = np.concatenate(outs, axis=1)
    return full.reshape(1, C, S, S, S).astype(np.float32)
